# revision 8
# baseline (speedup 1.0000x reference)
"""SincNet conv1d (KernelCustomFreezeConv) as a Bass/Tile TRN2 kernel.

Full inputs -> full output. Data-parallel over 8 NeuronCores: batch 32 is
sharded 4 per core; the 80x251 sinc filter bank is computed on host from the
three 80-scalar parameter vectors (tiny: ~20K flops) and replicated.

Per core the conv runs as matmuls on the tensor engine:
  out[f, t] = sum_k W[f,k] x[t+k],  k padded 251->256, split k = 128c+p.
  lhsT chunk c = W.T[128c:128c+128, :]  ([128, 80] stationary)
  rhs  chunk c = D[:, t0+128c : t0+128c+N] where D[p, m] = x[m+p] is a
  Toeplitz view materialized in SBUF by an overlapping-read DMA.
Both chunks accumulate into one PSUM bank; fp32r (1-pass fp32 matmul,
1 cycle/row at N>=256) gives ~4x the fp32 matmul throughput.
"""

import os
import numpy as np

FS = 16000.0
N_FILT = 80
FILT_DIM = 251
MIN_FREQ = 50.0

B_FULL = 32
L_IN = 32000
T_OUT = L_IN - FILT_DIM + 1  # 31750
N_CORES = 8
B_SHARD = B_FULL // N_CORES  # 4

KPAD = 256          # taps padded to 2*128
TILE_N = 512        # output cols per PSUM bank (fp32 max)
SEG_TILES = 8       # tiles per Toeplitz segment
SEG_COLS = TILE_N * SEG_TILES  # 2048
L_PAD = 32256       # padded x length so k in [251,256) reads zeros, not OOB

USE_F32R = False    # broken at runtime under axon; bf16 hi/lo split instead
USE_BF16_SPLIT = True

_cache = {}


def _build_filters(norm_f1, norm_f2, amplitude):
    """Mirror reference._build_filters in float32 numpy."""
    f32 = np.float32
    t_right = (np.linspace(1.0, (FILT_DIM - 1) / 2.0, (FILT_DIM - 1) // 2)
               .astype(f32) / f32(FS)).astype(f32)

    def sinc(band):
        arg = (2.0 * np.pi * band[:, None] * t_right[None, :]).astype(f32)
        y = (np.sin(arg) / arg).astype(f32)
        center = np.ones((band.shape[0], 1), dtype=f32)
        return np.concatenate([y[:, ::-1], center, y], axis=1)

    f1n = (np.abs(norm_f1) + f32(MIN_FREQ / FS)).astype(f32)
    f2n = (f1n + np.abs(norm_f2 - f1n) + f32(MIN_FREQ / FS)).astype(f32)
    f1 = (f1n * f32(FS)).astype(f32)
    f2 = (f2n * f32(FS)).astype(f32)
    amp = np.abs(amplitude).astype(f32)
    band = (amp[:, None] * (2.0 * f2[:, None] * sinc(f2)
                            - 2.0 * f1[:, None] * sinc(f1))).astype(f32)
    band = (band / band.max(axis=1, keepdims=True)).astype(f32)
    n = np.linspace(0.0, float(FILT_DIM), FILT_DIM).astype(f32)
    window = (0.54 - 0.46 * np.cos(2.0 * np.pi * n / FILT_DIM)).astype(f32)
    return (band * window[None, :]).astype(f32)  # [80, 251]


def _tiles_for(total):
    t0 = 0
    out = []
    while t0 < total:
        out.append((t0, min(TILE_N, total - t0)))
        t0 += TILE_N
    return out


def _build_program():
    import concourse.bacc as bacc
    import concourse.mybir as mybir
    from concourse import tile
    from concourse.ap import AP

    f32 = mybir.dt.float32
    bf16 = mybir.dt.bfloat16

    nc = bacc.Bacc("TRN2", target_bir_lowering=False, debug=False,
                   num_devices=N_CORES)
    # x hi/lo bf16 halves, interleaved as [2, B, L_PAD]: [0]=hi, [1]=lo
    x = nc.declare_dram_parameter("x", [2, B_SHARD, L_PAD], bf16,
                                  isOutput=False)
    # filters hi/lo: [2, KPAD, N_FILT]
    wt = nc.declare_dram_parameter("wt", [2, KPAD, N_FILT], bf16,
                                   isOutput=False)
    out = nc.declare_dram_parameter("out", [B_SHARD, N_FILT, T_OUT], f32,
                                    isOutput=True)

    with tile.TileContext(nc) as tc:
        with (
            tc.tile_pool(name="wpool", bufs=1) as wpool,
            tc.tile_pool(name="dpool", bufs=3) as dpool,
            tc.tile_pool(name="opool", bufs=3) as opool,
            tc.tile_pool(name="psum", bufs=8, space="PSUM") as psum_pool,
        ):
            w_sb = wpool.tile([128, 4 * N_FILT], bf16)
            # columns: [Wh0 | Wh1 | Wl0 | Wl1]
            for h in range(2):
                for c in range(2):
                    nc.sync.dma_start(
                        w_sb[:, (2 * h + c) * N_FILT:(2 * h + c + 1) * N_FILT],
                        wt[h][128 * c:128 * (c + 1), :])
            Wh = [w_sb[:, 0:N_FILT], w_sb[:, N_FILT:2 * N_FILT]]
            Wl = [w_sb[:, 2 * N_FILT:3 * N_FILT], w_sb[:, 3 * N_FILT:4 * N_FILT]]

            for b in range(B_SHARD):
                for s0 in range(0, T_OUT, SEG_COLS):
                    seg_cols = min(SEG_COLS, T_OUT - s0)
                    dw = seg_cols + 128
                    dh = dpool.tile([128, SEG_COLS + 128], bf16, tag="dhi")
                    dl = dpool.tile([128, SEG_COLS + 128], bf16, tag="dlo")
                    nc.sync.dma_start(
                        dh[:, :dw], AP(x, (0 * B_SHARD + b) * L_PAD + s0,
                                       [[1, 128], [1, dw]]))
                    nc.sync.dma_start(
                        dl[:, :dw], AP(x, (1 * B_SHARD + b) * L_PAD + s0,
                                       [[1, 128], [1, dw]]))
                    o_sb = opool.tile([128, SEG_COLS], f32, tag="oseg")
                    for (t0r, n) in _tiles_for(seg_cols):
                        ps = psum_pool.tile([128, TILE_N], f32)
                        hs = [dh[:, t0r:t0r + n], dh[:, t0r + 128:t0r + 128 + n]]
                        ls = [dl[:, t0r:t0r + n], dl[:, t0r + 128:t0r + 128 + n]]
                        # Wh.xh + Wh.xl + Wl.xh per chunk; Wl.xl dropped.
                        # Ordered so each stationary Wh chunk serves two
                        # consecutive matmuls.
                        nc.tensor.matmul(ps[:N_FILT, :n], Wh[0], hs[0],
                                         start=True, stop=False)
                        nc.tensor.matmul(ps[:N_FILT, :n], Wh[0], ls[0],
                                         start=False, stop=False)
                        nc.tensor.matmul(ps[:N_FILT, :n], Wh[1], hs[1],
                                         start=False, stop=False)
                        nc.tensor.matmul(ps[:N_FILT, :n], Wh[1], ls[1],
                                         start=False, stop=False)
                        nc.tensor.matmul(ps[:N_FILT, :n], Wl[0], hs[0],
                                         start=False, stop=False)
                        nc.tensor.matmul(ps[:N_FILT, :n], Wl[1], hs[1],
                                         start=False, stop=True)
                        nc.vector.tensor_copy(o_sb[:N_FILT, t0r:t0r + n],
                                              ps[:N_FILT, :n])
                    nc.scalar.dma_start(out[b][:, s0:s0 + seg_cols],
                                        o_sb[:N_FILT, :seg_cols])
    nc.finalize()
    return nc


def _get_program():
    if "nc" not in _cache:
        _cache["nc"] = _build_program()
    return _cache["nc"]


def kernel(x, norm_f1, norm_f2, amplitude, _trace=False):
    from concourse.bass_utils import run_bass_kernel_spmd

    x = np.asarray(x, dtype=np.float32)
    W = _build_filters(np.asarray(norm_f1, np.float32),
                       np.asarray(norm_f2, np.float32),
                       np.asarray(amplitude, np.float32))
    wt = np.zeros((KPAD, N_FILT), dtype=np.float32)
    wt[:FILT_DIM, :] = W.T

    import ml_dtypes
    bf = ml_dtypes.bfloat16
    wt_hi = wt.astype(bf)
    wt_lo = (wt - wt_hi.astype(np.float32)).astype(bf)
    wt2 = np.stack([wt_hi, wt_lo])  # [2, 256, 80]

    xs = x.reshape(B_FULL, L_IN)
    in_maps = []
    for c in range(N_CORES):
        shard = xs[c * B_SHARD:(c + 1) * B_SHARD]
        xp = np.zeros((B_SHARD, L_PAD), dtype=np.float32)
        xp[:, :L_IN] = shard
        x_hi = xp.astype(bf)
        x_lo = (xp - x_hi.astype(np.float32)).astype(bf)
        in_maps.append({"x": np.stack([x_hi, x_lo]), "wt": wt2})

    nc = _get_program()
    res = run_bass_kernel_spmd(nc, in_maps, list(range(N_CORES)))
    outs = [res.results[c]["out"] for c in range(N_CORES)]
    full = np.concatenate(outs, axis=0)  # [32, 80, 31750]
    if _trace:
        _cache["last_result"] = res
    return full
